# revision 12
# baseline (speedup 1.0000x reference)
"""Bahdanau attention kernel for Trainium2 (Bass/Tile), data-parallel over batch.

Problem (full shapes):
    encoder_output   [S=2048, B=16, H=1024] f32
    last_decoder_state [2, 1, B, H] f32   (only [0,0] used -> state [B, H])
    W [H, H], b [H]
    energy = state @ W.T + b                  [B, H]
    scores = einsum('sbh,bh->sb', enc, energy) [S, B]
    out    = softmax(scores, axis=0)[None, None]  [1, 1, S, B]

Sharding: batch split across 8 cores (2 encoder batches each). W is sharded
by output rows (128 j-rows per core): each core computes an energy partial
[128, 16] for ALL batches, an AllGather rebuilds the full energy [1024, 16]
everywhere (8 KB/core on the wire vs reading the whole replicated 4 MB W per
core from HBM). Scores are then computed for the local 2 encoder batches
against all 16 energies (PE time is set by the moving operand, not the
stationary width), so the program is identical on every core; the host picks
the matching row. Softmax is over S which is fully resident per core.

Per-core device program:
    epart[j_loc, b] = sum_i W[j,i] state[b,i] + bias[j]   j in core's 128 rows
    energy = AllGather(epart)                              [1024, 16]
    scores[e, s]  = sum_h energy[h, e] * enc[b, h, s]      for b in {0,1}
    probs = softmax over s  -> [2, 16, S], host keeps row 2c+b of block b

Host-side prep: per-core batch slice of enc transposed to [b, h, s], W^T
column slice, bias slice; every DMA reads long contiguous rows.

`reps`/`dynamic` exist only for benchmarking: they repeat the body inside one
NEFF (statically unrolled or as a Tile For_i loop) so HW time can be measured
through a high-latency dispatch path. kernel() always uses reps=1.
"""

import numpy as np

S, B, H = 2048, 16, 1024
NCORES = 8
BL = B // NCORES  # 2 batches per core
P = 128           # partitions
HT = H // P       # 8 h-tiles
SCW = 512         # matmul moving-operand max (one PSUM bank of f32)
SC = S // SCW     # 4 seq chunks

_cached = {}


def _build_nc(reps=1, dynamic=False):
    import concourse.bacc as bacc
    import concourse.bass as bass
    import concourse.tile as tile
    from concourse import mybir

    f32 = mybir.dt.float32
    nc = bacc.Bacc("TRN2", target_bir_lowering=False, debug=False, num_devices=NCORES)

    enc = nc.dram_tensor("enc", [BL, H, S], f32, kind="ExternalInput").ap()
    state_t = nc.dram_tensor("state_t", [H, B], f32, kind="ExternalInput").ap()
    w_slice_t = nc.dram_tensor("w_slice_t", [H, P], f32, kind="ExternalInput").ap()
    bias_sl = nc.dram_tensor("bias_sl", [P], f32, kind="ExternalInput").ap()
    probs = nc.dram_tensor("probs", [BL, B, S], f32, kind="ExternalOutput").ap()

    with tile.TileContext(nc) as tc:
        with (
            tc.tile_pool(name="consts", bufs=min(2, reps)) as consts,
            tc.tile_pool(name="encpool", bufs=4) as encpool,
            tc.tile_pool(name="pe_ps", bufs=2, space=bass.MemorySpace.PSUM) as pe_pool,
            tc.tile_pool(name="sc_ps", bufs=1, space=bass.MemorySpace.PSUM) as ps_pool,
            tc.tile_pool(name="spool", bufs=min(2, reps)) as spool,
            tc.tile_pool(name="ccpool", bufs=min(2, reps), space="DRAM") as ccpool,
        ):

            def emit_rep():
                # small operands ride the SWDGE stream so the two HWDGE rings
                # are free for the big encoder reads (three concurrent DMA
                # streams round-robin at packet granularity on the SDMAs)
                ws = consts.tile([P, HT, P], f32)     # W^T slice [i, (it, j)]
                nc.gpsimd.dma_start(
                    out=ws[:], in_=w_slice_t.rearrange("(t p) j -> p t j", p=P)
                )
                st = consts.tile([P, HT, B], f32)     # state^T [i, (it, b)]
                nc.gpsimd.dma_start(
                    out=st[:], in_=state_t.rearrange("(t p) b -> p t b", p=P)
                )
                bsl = consts.tile([P, 1], f32)        # bias slice [j_loc, 1]
                nc.gpsimd.dma_start(
                    out=bsl[:], in_=bias_sl.rearrange("(t p) -> p t", p=P)
                )

                # energy partial for this core's 128 j-rows, all 16 batches
                pe = pe_pool.tile([P, B], f32)
                for it in range(HT):
                    nc.tensor.matmul(
                        pe[:],
                        ws[:, it, :],   # lhsT [i, j_loc]
                        st[:, it, :],   # rhs  [i, b]
                        start=(it == 0),
                        stop=(it == HT - 1),
                    )
                epart = consts.tile([P, B], f32)
                nc.scalar.activation(
                    out=epart[:],
                    in_=pe[:],
                    func=mybir.ActivationFunctionType.Identity,
                    bias=bsl[:],
                    scale=1.0,
                )

                # AllGather energy partials -> full energy [1024, 16]
                cc_in = ccpool.tile([P, B], f32)
                cc_out = ccpool.tile([H, B], f32, addr_space="Shared")
                nc.gpsimd.dma_start(out=cc_in[:], in_=epart[:])
                nc.gpsimd.collective_compute(
                    "AllGather",
                    mybir.AluOpType.bypass,
                    replica_groups=[list(range(NCORES))],
                    ins=[cc_in[:]],
                    outs=[cc_out[:]],
                )
                egy = consts.tile([P, HT, B], f32)    # energy [h, (ht, e)]
                nc.gpsimd.dma_start(
                    out=egy[:], in_=cc_out.rearrange("(t p) b -> p t b", p=P)
                )

                # scores vs all 16 energies: psum block b=0 at partition 0,
                # b=1 at partition 32 (matmul base partition must be 0/32/64)
                ps = ps_pool.tile([32 + B, S], f32)
                hwdge = [nc.sync, nc.scalar]  # two independent HWDGE rings
                for b in range(BL):
                    for ht in range(HT):
                        et = encpool.tile([P, S], f32)
                        hwdge[(b * HT + ht) % 2].dma_start(
                            out=et[:], in_=enc[b, ht * P:(ht + 1) * P, :]
                        )
                        for sc in range(SC):
                            nc.tensor.matmul(
                                ps[32 * b:32 * b + B, sc * SCW:(sc + 1) * SCW],
                                egy[:, ht, :],                   # lhsT [h, e]
                                et[:, sc * SCW:(sc + 1) * SCW],  # rhs [h, s]
                                start=(ht == 0),
                                stop=(ht == HT - 1),
                            )

                # softmax over s (free dim), per batch block; every op's APs
                # share the same base partition so engine lanes stay aligned
                prob_sb = spool.tile([32 + B, S], f32)
                nmax = spool.tile([32 + B, 1], f32)
                ssum = spool.tile([32 + B, 1], f32)
                rinv = spool.tile([32 + B, 1], f32)
                for b in range(BL):
                    r = 32 * b
                    nc.vector.reduce_max(
                        nmax[r:r + B, :], ps[r:r + B, :],
                        axis=mybir.AxisListType.X, negate=True,
                    )
                    nc.scalar.activation(
                        out=prob_sb[r:r + B, :],
                        in_=ps[r:r + B, :],
                        func=mybir.ActivationFunctionType.Exp,
                        bias=nmax[r:r + B, :],
                        scale=1.0,
                        accum_out=ssum[r:r + B, :],
                    )
                    nc.vector.reciprocal(rinv[r:r + B, :], ssum[r:r + B, :])
                    nc.vector.tensor_scalar_mul(
                        out=prob_sb[r:r + B, :],
                        in0=prob_sb[r:r + B, :],
                        scalar1=rinv[r:r + B, :],
                    )
                    nc.sync.dma_start(
                        out=probs[b], in_=prob_sb[r:r + B, :]
                    )

            if dynamic and reps > 1:
                with tc.For_i(0, reps, 1):
                    emit_rep()
            else:
                for _rep in range(reps):
                    emit_rep()

    nc.compile()
    return nc


def get_nc(reps=1, dynamic=False):
    key = ("nc", reps, dynamic)
    if key not in _cached:
        _cached[key] = _build_nc(reps, dynamic)
    return _cached[key]


def prep_in_maps(encoder_output, last_decoder_state, W, b):
    enc = np.asarray(encoder_output, dtype=np.float32)
    state = np.asarray(last_decoder_state, dtype=np.float32)[0, 0]  # [B, H]
    st_t = np.ascontiguousarray(state.T)                            # [H, B]
    Wt = np.ascontiguousarray(np.asarray(W, dtype=np.float32).T)    # [i, j]
    bias = np.ascontiguousarray(np.asarray(b, dtype=np.float32))
    in_maps = []
    for c in range(NCORES):
        b0 = BL * c
        j0 = P * c
        in_maps.append({
            "enc": np.ascontiguousarray(enc[:, b0:b0 + BL, :].transpose(1, 2, 0)),
            "state_t": st_t,
            "w_slice_t": np.ascontiguousarray(Wt[:, j0:j0 + P]),
            "bias_sl": np.ascontiguousarray(bias[j0:j0 + P]),
        })
    return in_maps


def assemble(results):
    out = np.empty((S, B), np.float32)
    for c in range(NCORES):
        for b in range(BL):
            bg = BL * c + b
            out[:, bg] = results[c]["probs"][b, bg, :]
    return out[None, None]


def kernel(encoder_output, last_decoder_state, W, b):
    from concourse.bass_utils import run_bass_kernel_spmd

    nc = get_nc()
    in_maps = prep_in_maps(encoder_output, last_decoder_state, W, b)
    res = run_bass_kernel_spmd(nc, in_maps, core_ids=list(range(NCORES)))
    return assemble(res.results)


# revision 17
# speedup vs baseline: 1.6283x; 1.6283x over previous
"""Bahdanau attention kernel for Trainium2 (Bass/Tile), data-parallel over batch.

Problem (full shapes):
    encoder_output   [S=2048, B=16, H=1024] f32
    last_decoder_state [2, 1, B, H] f32   (only [0,0] used -> state [B, H])
    W [H, H], b [H]
    energy = state @ W.T + b                  [B, H]
    scores = einsum('sbh,bh->sb', enc, energy) [S, B]
    out    = softmax(scores, axis=0)[None, None]  [1, 1, S, B]

Sharding: batch split across 8 cores (2 encoder batches each). W is sharded
by output rows (128 j-rows per core): each core computes an energy partial
[128, 16] for ALL batches, an AllGather rebuilds the full energy [1024, 16]
everywhere (8 KB/core on the wire vs reading the whole replicated 4 MB W per
core from HBM). Scores are then computed for the local 2 encoder batches
against all 16 energies (PE time is set by the moving operand, not the
stationary width), so the program is identical on every core; the host picks
the matching row. Softmax is over S which is fully resident per core.

Per-core device program:
    epart[j_loc, b] = sum_i W[j,i] state[b,i] + bias[j]   j in core's 128 rows
    energy = AllGather(epart)                              [1024, 16]
    scores[e, s]  = sum_h energy[h, e] * enc[b, h, s]      for b in {0,1}
    probs = softmax over s  -> [2, 16, S], host keeps row 2c+b of block b

Host-side prep: per-core batch slice of enc transposed to [b, h, s], W^T
column slice, bias slice; every DMA reads long contiguous rows.

`reps`/`dynamic` exist only for benchmarking: they repeat the body inside one
NEFF (statically unrolled or as a Tile For_i loop) so HW time can be measured
through a high-latency dispatch path. kernel() always uses reps=1.
"""

import numpy as np

S, B, H = 2048, 16, 1024
NCORES = 8
BL = B // NCORES  # 2 batches per core
P = 128           # partitions
HT = H // P       # 8 h-tiles
SCW = 512         # matmul moving-operand max (one PSUM bank of f32)
SC = S // SCW     # 4 seq chunks

_cached = {}


def _build_nc(reps=1, dynamic=False, variant="cc"):
    import concourse.bacc as bacc
    import concourse.bass as bass
    import concourse.tile as tile
    from concourse import mybir

    f32 = mybir.dt.float32
    nc = bacc.Bacc("TRN2", target_bir_lowering=False, debug=False, num_devices=NCORES)

    enc = nc.dram_tensor("enc", [BL, H, S], f32, kind="ExternalInput").ap()
    if variant == "cc":
        state_t = nc.dram_tensor("state_t", [H, B], f32, kind="ExternalInput").ap()
        w_slice_t = nc.dram_tensor("w_slice_t", [H, P], f32, kind="ExternalInput").ap()
        bias_sl = nc.dram_tensor("bias_sl", [P], f32, kind="ExternalInput").ap()
    else:  # "nocc" diagnostic: energy precomputed on host
        energy_t = nc.dram_tensor("energy_t", [H, B], f32, kind="ExternalInput").ap()
    probs = nc.dram_tensor("probs", [BL, B, S], f32, kind="ExternalOutput").ap()

    with tile.TileContext(nc) as tc:
        with (
            tc.tile_pool(name="consts", bufs=min(2, reps)) as consts,
            tc.tile_pool(name="encpool", bufs=4) as encpool,
            tc.tile_pool(name="pe_ps", bufs=2, space=bass.MemorySpace.PSUM) as pe_pool,
            tc.tile_pool(name="sc_ps", bufs=1, space=bass.MemorySpace.PSUM) as ps_pool,
            tc.tile_pool(name="spool", bufs=min(2, reps)) as spool,
            tc.tile_pool(name="ccpool", bufs=min(2, reps), space="DRAM") as ccpool,
        ):

            def emit_rep_nocc():
                egy = consts.tile([P, HT, B], f32)    # energy [h, (ht, e)]
                nc.gpsimd.dma_start(
                    out=egy[:], in_=energy_t.rearrange("(t p) b -> p t b", p=P)
                )
                return egy

            def emit_rep_cc():
                # small operands ride the SWDGE stream so the two HWDGE rings
                # are free for the big encoder reads (three concurrent DMA
                # streams round-robin at packet granularity on the SDMAs)
                ws = consts.tile([P, HT, P], f32)     # W^T slice [i, (it, j)]
                nc.gpsimd.dma_start(
                    out=ws[:], in_=w_slice_t.rearrange("(t p) j -> p t j", p=P)
                )
                st = consts.tile([P, HT, B], f32)     # state^T [i, (it, b)]
                nc.gpsimd.dma_start(
                    out=st[:], in_=state_t.rearrange("(t p) b -> p t b", p=P)
                )
                bsl = consts.tile([P, 1], f32)        # bias slice [j_loc, 1]
                nc.gpsimd.dma_start(
                    out=bsl[:], in_=bias_sl.rearrange("(t p) -> p t", p=P)
                )

                # energy partial for this core's 128 j-rows, all 16 batches
                pe = pe_pool.tile([P, B], f32)
                for it in range(HT):
                    nc.tensor.matmul(
                        pe[:],
                        ws[:, it, :],   # lhsT [i, j_loc]
                        st[:, it, :],   # rhs  [i, b]
                        start=(it == 0),
                        stop=(it == HT - 1),
                    )
                epart = consts.tile([P, B], f32)
                nc.scalar.activation(
                    out=epart[:],
                    in_=pe[:],
                    func=mybir.ActivationFunctionType.Identity,
                    bias=bsl[:],
                    scale=1.0,
                )

                # AllGather energy partials -> full energy [1024, 16]
                cc_in = ccpool.tile([P, B], f32)
                cc_out = ccpool.tile([H, B], f32, addr_space="Shared")
                nc.gpsimd.dma_start(out=cc_in[:], in_=epart[:])
                nc.gpsimd.collective_compute(
                    "AllGather",
                    mybir.AluOpType.bypass,
                    replica_groups=[list(range(NCORES))],
                    ins=[cc_in[:]],
                    outs=[cc_out[:]],
                )
                egy = consts.tile([P, HT, B], f32)    # energy [h, (ht, e)]
                nc.gpsimd.dma_start(
                    out=egy[:], in_=cc_out.rearrange("(t p) b -> p t b", p=P)
                )
                return egy

            def emit_rep():
                egy = emit_rep_cc() if variant == "cc" else emit_rep_nocc()

                # scores vs all 16 energies: psum block b=0 at partition 0,
                # b=1 at partition 32 (matmul base partition must be 0/32/64)
                ps = ps_pool.tile([32 + B, S], f32)
                hwdge = [nc.sync, nc.scalar]  # two independent HWDGE rings
                for b in range(BL):
                    for ht in range(HT):
                        et = encpool.tile([P, S], f32)
                        hwdge[(b * HT + ht) % 2].dma_start(
                            out=et[:], in_=enc[b, ht * P:(ht + 1) * P, :]
                        )
                        for sc in range(SC):
                            nc.tensor.matmul(
                                ps[32 * b:32 * b + B, sc * SCW:(sc + 1) * SCW],
                                egy[:, ht, :],                   # lhsT [h, e]
                                et[:, sc * SCW:(sc + 1) * SCW],  # rhs [h, s]
                                start=(ht == 0),
                                stop=(ht == HT - 1),
                            )

                # softmax over s (free dim), per batch block; every op's APs
                # share the same base partition so engine lanes stay aligned
                prob_sb = spool.tile([32 + B, S], f32)
                nmax = spool.tile([32 + B, 1], f32)
                ssum = spool.tile([32 + B, 1], f32)
                rinv = spool.tile([32 + B, 1], f32)
                for b in range(BL):
                    r = 32 * b
                    nc.vector.reduce_max(
                        nmax[r:r + B, :], ps[r:r + B, :],
                        axis=mybir.AxisListType.X, negate=True,
                    )
                    nc.scalar.activation(
                        out=prob_sb[r:r + B, :],
                        in_=ps[r:r + B, :],
                        func=mybir.ActivationFunctionType.Exp,
                        bias=nmax[r:r + B, :],
                        scale=1.0,
                        accum_out=ssum[r:r + B, :],
                    )
                    nc.vector.reciprocal(rinv[r:r + B, :], ssum[r:r + B, :])
                    nc.vector.tensor_scalar_mul(
                        out=prob_sb[r:r + B, :],
                        in0=prob_sb[r:r + B, :],
                        scalar1=rinv[r:r + B, :],
                    )
                    nc.sync.dma_start(
                        out=probs[b], in_=prob_sb[r:r + B, :]
                    )

            if dynamic and reps > 1:
                with tc.For_i(0, reps, 1):
                    emit_rep()
            else:
                for _rep in range(reps):
                    emit_rep()

    nc.compile()
    return nc


def get_nc(reps=1, dynamic=False, variant="cc"):
    key = ("nc", reps, dynamic, variant)
    if key not in _cached:
        _cached[key] = _build_nc(reps, dynamic, variant)
    return _cached[key]


def prep_in_maps(encoder_output, last_decoder_state, W, b, variant="cc"):
    enc = np.asarray(encoder_output, dtype=np.float32)
    state = np.asarray(last_decoder_state, dtype=np.float32)[0, 0]  # [B, H]
    st_t = np.ascontiguousarray(state.T)                            # [H, B]
    Wt = np.ascontiguousarray(np.asarray(W, dtype=np.float32).T)    # [i, j]
    bias = np.ascontiguousarray(np.asarray(b, dtype=np.float32))
    if variant != "cc":
        energy_t = np.ascontiguousarray(
            (state @ np.asarray(W, dtype=np.float32).T + bias).T
        )  # [H, B]
    in_maps = []
    for c in range(NCORES):
        b0 = BL * c
        j0 = P * c
        m = {"enc": np.ascontiguousarray(enc[:, b0:b0 + BL, :].transpose(1, 2, 0))}
        if variant == "cc":
            m["state_t"] = st_t
            m["w_slice_t"] = np.ascontiguousarray(Wt[:, j0:j0 + P])
            m["bias_sl"] = np.ascontiguousarray(bias[j0:j0 + P])
        else:
            m["energy_t"] = energy_t
        in_maps.append(m)
    return in_maps


def assemble(results):
    out = np.empty((S, B), np.float32)
    for c in range(NCORES):
        for b in range(BL):
            bg = BL * c + b
            out[:, bg] = results[c]["probs"][b, bg, :]
    return out[None, None]


def kernel(encoder_output, last_decoder_state, W, b):
    from concourse.bass_utils import run_bass_kernel_spmd

    nc = get_nc()
    in_maps = prep_in_maps(encoder_output, last_decoder_state, W, b)
    res = run_bass_kernel_spmd(nc, in_maps, core_ids=list(range(NCORES)))
    return assemble(res.results)
